# revision 1
# baseline (speedup 1.0000x reference)
"""CGRUCell Trainium2 kernel — hybrid: data-parallel x4 over batch,
tensor-parallel x2 over units, on 8 NeuronCores.

Core c: batch group g=c//2 (256 rows), unit parity p=c%2 (units
[p*1024:(p+1)*1024], i.e. its real+imag output columns). Weights are
split by parity (100.7 MB/core instead of 201 MB replicated). The only
cross-core data is r*h for the candidate gate: a pairwise AllGather,
overlapped with gate-z matmuls.

Gate r is computed output-transposed (weights stationary, activations
moving) so r and r*h are produced directly in K-major layout — no
on-chip transposes. Gates z and h use the batch-stationary orientation
(N=512 moving) for better weight-load amortization.

Matmuls run in float32r (TF32-like full-rate fp32 mode, rel err ~1e-4).
Gate order: r -> z -> h so the r*h exchange hides under z's compute.
"""

import sys

for _p in ("/opt/trn_rl_repo", "/root/.axon_site/_ro/trn_rl_repo"):
    if _p not in sys.path:
        sys.path.append(_p)

import numpy as np

import concourse.bass as bass
import concourse.mybir as mybir
import concourse.tile as tile
from concourse import bacc
from concourse.bass_utils import run_bass_kernel_spmd

P = 128
U = 2048           # UNITS
B = 1024
N_CORES = 8
GROUPS = 4
BC = B // GROUPS   # 256 batch rows per core
MSUB = BC // P     # 2 m-subtiles
UC = U // 2        # 1024 unit columns per core (per half)
KCH = U // P       # 16 k-chunks per complex half
F32 = mybir.dt.float32
MM_DT = mybir.dt.float32r
NBLK = UC // 512   # 2 col-blocks of 512 per half
BLK = 512

_CACHE = {}


def _build_nc(repeat=1):
    nc = bacc.Bacc(None, target_bir_lowering=False)

    # K-major activations (full contraction dims, batch 256 of this group)
    x1 = nc.dram_tensor("x1", [P, KCH, BC], MM_DT, kind="ExternalInput")
    x2 = nc.dram_tensor("x2", [P, KCH, BC], MM_DT, kind="ExternalInput")
    h1 = nc.dram_tensor("h1", [P, KCH, BC], MM_DT, kind="ExternalInput")
    h2 = nc.dram_tensor("h2", [P, KCH, BC], MM_DT, kind="ExternalInput")
    # batch-major h_tm1, own columns only: [256, 2048]
    hbm = nc.dram_tensor("hbm", [BC, 2 * UC], F32, kind="ExternalInput")
    # K-major h_tm1, own columns: [p, o(8 re + 8 im), b]
    hTo = nc.dram_tensor("hTo", [P, KCH, BC], MM_DT, kind="ExternalInput")
    # weights, parity-sliced on host: [2048, 3*1024] (gate z|r|h blocks)
    RK = nc.dram_tensor("RK", [U, 3 * UC], MM_DT, kind="ExternalInput")
    IK = nc.dram_tensor("IK", [U, 3 * UC], MM_DT, kind="ExternalInput")
    RR = nc.dram_tensor("RR", [U, 3 * UC], MM_DT, kind="ExternalInput")
    IR = nc.dram_tensor("IR", [U, 3 * UC], MM_DT, kind="ExternalInput")
    # bias for z/h, own columns, block order [gate(3), half(2), 1024]
    # (gate r entries unused, kept for layout simplicity)
    bias = nc.dram_tensor("bias", [3 * 2 * UC], F32, kind="ExternalInput")
    # gate-r bias, column-major pre-scaled: b' = 0.2*b + 0.5, [128, 16]
    biasr = nc.dram_tensor("biasr", [P, KCH], F32, kind="ExternalInput")
    out = nc.dram_tensor("out", [BC, 2 * UC], F32, kind="ExternalOutput")

    hbm_r = hbm.rearrange("(m p) c -> p m c", p=P)
    out_r = out.rearrange("(m p) c -> p m c", p=P)

    with tile.TileContext(nc) as tc:
        with (
            tc.tile_pool(name="acts", bufs=1) as acts,
            tc.tile_pool(name="wpool", bufs=2) as wpool,
            tc.tile_pool(name="psum", bufs=1, space="PSUM") as psum,
            tc.tile_pool(name="small", bufs=2) as small,
            tc.tile_pool(name="neg", bufs=2) as negp,
            tc.tile_pool(name="bigs", bufs=1) as bigs,
            tc.tile_pool(name="dram", bufs=1, space="DRAM") as dram,
        ):
            x1s = acts.tile([P, KCH, BC], MM_DT, tag="x1s", name="x1s")
            x2s = acts.tile([P, KCH, BC], MM_DT, tag="x2s", name="x2s")
            h1s = acts.tile([P, KCH, BC], MM_DT, tag="h1s", name="h1s")
            h2s = acts.tile([P, KCH, BC], MM_DT, tag="h2s", name="h2s")
            hTos = acts.tile([P, KCH, BC], MM_DT, tag="hTos", name="hTos")
            rh2s = acts.tile([P, KCH, BC], MM_DT, tag="rh2s", name="rh2s")
            # chunked act loads: the first k-slices land in ~2us so the
            # gate-r matmuls start without waiting for the full 10 MB
            for o in range(0, KCH, 4):
                sl = slice(o, o + 4)
                nc.sync.dma_start(x1s[:, sl, :], x1[:, sl, :])
                nc.sync.dma_start(x2s[:, sl, :], x2[:, sl, :])
                nc.sync.dma_start(h1s[:, sl, :], h1[:, sl, :])
                nc.sync.dma_start(h2s[:, sl, :], h2[:, sl, :])
                nc.sync.dma_start(hTos[:, sl, :], hTo[:, sl, :])

            hbmt = bigs.tile([P, MSUB, 2 * UC], F32, tag="hbmt", name="hbmt")
            nc.sync.dma_start(hbmt[:], hbm_r)

            z_sb = bigs.tile([P, MSUB, 2 * UC], F32, tag="z_sb", name="z_sb")
            hh_sb = bigs.tile([P, MSUB, 2 * UC], F32, tag="hh_sb", name="hh_sb")

            brcol = small.tile([P, KCH], F32, tag="brcol", name="brcol", bufs=1)
            nc.sync.dma_start(brcol[:], biasr[:])

            def wtile(rep, g, tname, k, bb, src, width=BLK):
                t = wpool.tile([P, BLK], MM_DT, tag=tname,
                               name=f"{tname}_{rep}_{g}_{k}_{bb}")
                ksl = slice(k * P, (k + 1) * P)
                csl = slice(g * UC + bb * width, g * UC + (bb + 1) * width)
                nc.sync.dma_start(t[:], src[ksl, csl])
                return t

            def gate_phase_a(rep, g, b1, b2, evict):
                """Batch-stationary gates (z, h): 8 psum blocks
                [m, half, bb] of [128 batch, 512 cols]."""
                ps = {}
                for m in range(MSUB):
                    for half in range(2):
                        for bb in range(NBLK):
                            ps[(m, half, bb)] = psum.tile(
                                [P, BLK], F32,
                                tag=f"ps{(m * 2 + half) * NBLK + bb}",
                                name=f"ps_{rep}_{g}_{m}_{half}_{bb}",
                            )
                for k in range(KCH):
                    wts = {
                        n: [wtile(rep, g, f"{n}{bb}", k, bb, src)
                            for bb in range(NBLK)]
                        for n, src in (("rk", RK), ("ik", IK),
                                       ("rr", RR), ("ir", IR))
                    }
                    na1 = negp.tile([P, BC], MM_DT, tag="na1",
                                    name=f"na1_{rep}_{g}_{k}")
                    nb1 = negp.tile([P, BC], MM_DT, tag="nb1",
                                    name=f"nb1_{rep}_{g}_{k}")
                    nc.vector.tensor_scalar(
                        na1[:], x1s[:, k, :], -1.0, None, mybir.AluOpType.mult
                    )
                    nc.vector.tensor_scalar(
                        nb1[:], b1[:, k, :], -1.0, None, mybir.AluOpType.mult
                    )

                    first = k == 0
                    last = k == KCH - 1
                    streams = [
                        (x1s, "rk", 0, first, False),
                        (x2s, "rk", 1, first, False),
                        (x2s, "ik", 0, False, False),
                        (na1, "ik", 1, False, False),
                        (b1, "rr", 0, False, False),
                        (b2, "rr", 1, False, False),
                        (b2, "ir", 0, False, last),
                        (nb1, "ir", 1, False, last),
                    ]
                    for stat, wn, half, st, sp in streams:
                        for m in range(MSUB):
                            if stat is na1 or stat is nb1:
                                lhsT = stat[:, m * P : (m + 1) * P]
                            else:
                                lhsT = stat[:, k, m * P : (m + 1) * P]
                            for bb in range(NBLK):
                                nc.tensor.matmul(
                                    ps[(m, half, bb)],
                                    lhsT,
                                    wts[wn][bb][:],
                                    start=st,
                                    stop=sp,
                                )

                for half in range(2):
                    for bb in range(NBLK):
                        j = half * NBLK + bb
                        bt = small.tile([P, BLK], F32, tag="bt",
                                        name=f"bt_{rep}_{g}_{j}")
                        nc.sync.dma_start(
                            bt[:],
                            bias[None, g * 2 * UC + j * BLK : g * 2 * UC
                                 + (j + 1) * BLK].to_broadcast((P, BLK)),
                        )
                        for m in range(MSUB):
                            oc = slice(half * UC + bb * BLK,
                                       half * UC + (bb + 1) * BLK)
                            evict(ps[(m, half, bb)], bt, m, oc)

            def evict_hs(dest):
                def _e(pst, bt, m, oc):
                    d = dest[:, m, oc]
                    nc.vector.tensor_add(d, pst[:], bt[:])
                    nc.vector.tensor_scalar(
                        d, d, 0.2, 0.5,
                        mybir.AluOpType.mult, mybir.AluOpType.add,
                    )
                    nc.vector.tensor_scalar(
                        d, d, 1.0, 0.0,
                        mybir.AluOpType.min, mybir.AluOpType.max,
                    )
                return _e

            def evict_tanh(dest):
                def _e(pst, bt, m, oc):
                    tmp = small.tile([P, BLK], F32, tag="ttmp", name="ttmp")
                    nc.vector.tensor_add(tmp[:], pst[:], bt[:])
                    nc.scalar.activation(
                        dest[:, m, oc], tmp[:], mybir.ActivationFunctionType.Tanh
                    )
                return _e

            def gate_r_transposed(rep, rhTl):
                """Gate r, output-transposed: psum [128 cols, 256 batch].
                grp 0 covers weight cols 0:512 (real ccs 0-3 + imag ccs 0-3),
                grp 1 covers cols 512:1024. Writes rhT = hs(pre_r)*hT
                directly into rhTl[:, ccg, :]."""
                g = 1
                for grp in range(2):
                    ps = [
                        psum.tile([P, BC], F32, tag=f"ps{i}",
                                  name=f"psr_{rep}_{grp}_{i}")
                        for i in range(8)
                    ]
                    # ps[0..3] real ccs, ps[4..7] imag ccs
                    for k in range(KCH):
                        wts = {
                            n: wtile(rep, g, f"{n}{grp}", k, grp, src)
                            for n, src in (("rk", RK), ("ik", IK),
                                           ("rr", RR), ("ir", IR))
                        }
                        na1 = negp.tile([P, BC], MM_DT, tag="na1",
                                        name=f"na1r_{rep}_{grp}_{k}")
                        nb1 = negp.tile([P, BC], MM_DT, tag="nb1",
                                        name=f"nb1r_{rep}_{grp}_{k}")
                        nc.vector.tensor_scalar(
                            na1[:], x1s[:, k, :], -1.0, None,
                            mybir.AluOpType.mult,
                        )
                        nc.vector.tensor_scalar(
                            nb1[:], h1s[:, k, :], -1.0, None,
                            mybir.AluOpType.mult,
                        )
                        first = k == 0
                        last = k == KCH - 1
                        # (weight, moving, psum base, start, stop)
                        streams = [
                            ("rk", x1s, 0, first, False),
                            ("rk", x2s, 4, first, False),
                            ("ik", x2s, 0, False, False),
                            ("ik", na1, 4, False, False),
                            ("rr", h1s, 0, False, False),
                            ("rr", h2s, 4, False, False),
                            ("ir", h2s, 0, False, last),
                            ("ir", nb1, 4, False, last),
                        ]
                        for wn, mov, base, st, sp in streams:
                            if mov is na1 or mov is nb1:
                                rhs = mov[:]
                            else:
                                rhs = mov[:, k, :]
                            for cc in range(4):
                                nc.tensor.matmul(
                                    ps[base + cc],
                                    wts[wn][:, cc * P : (cc + 1) * P],
                                    rhs,
                                    start=st,
                                    stop=sp,
                                )
                    # evict: rhT[ccg] = clip(0.2*psum + b') * hT_own[ccg]
                    for i in range(8):
                        half = i // 4          # 0 real, 1 imag
                        ccg = half * 8 + grp * 4 + (i % 4)
                        d = rhTl[:, ccg, :]
                        nc.vector.tensor_scalar(
                            d, ps[i][:], 0.2, brcol[:, ccg : ccg + 1],
                            mybir.AluOpType.mult, mybir.AluOpType.add,
                        )
                        nc.vector.tensor_scalar(
                            d, d, 1.0, 0.0,
                            mybir.AluOpType.min, mybir.AluOpType.max,
                        )
                        nc.vector.tensor_mul(d, d, hTos[:, ccg, :])

            for rep in range(repeat):
                # --- gate r first (g=1), output-transposed ---
                rhTl = acts.tile([P, KCH, BC], MM_DT, tag="rh1s",
                                 name=f"rhTl_{rep}")
                gate_r_transposed(rep, rhTl)

                # pairwise AllGather of rhT
                inb = dram.tile([P, KCH, BC], MM_DT, tag="inb",
                                name=f"inb_{rep}")
                outb = dram.tile([2, P, KCH, BC], MM_DT, tag="outb",
                                 name=f"outb_{rep}")
                nc.sync.dma_start(inb[:], rhTl[:])
                nc.gpsimd.collective_compute(
                    "AllGather",
                    mybir.AluOpType.bypass,
                    replica_groups=[[0, 1], [2, 3], [4, 5], [6, 7]],
                    ins=[inb[:].opt()],
                    outs=[outb[:].opt()],
                )
                rh1s = acts.tile([P, KCH, BC], MM_DT, tag="rh1s",
                                 name=f"rh1s_{rep}")
                # real rows: parity0 units 0:1024 -> o 0..7, parity1 -> 8..15
                nc.sync.dma_start(rh1s[:, 0:8, :], outb[0, :, 0:8, :])
                nc.sync.dma_start(rh1s[:, 8:16, :], outb[1, :, 0:8, :])
                nc.sync.dma_start(rh2s[:, 0:8, :], outb[0, :, 8:16, :])
                nc.sync.dma_start(rh2s[:, 8:16, :], outb[1, :, 8:16, :])

                # --- gate z (g=0), overlaps with the collective ---
                gate_phase_a(rep, 0, h1s, h2s, evict_hs(z_sb))

                # --- gate h (g=2) ---
                gate_phase_a(rep, 2, rh1s, rh2s, evict_tanh(hh_sb))

                # h_new = hh + z*(h - hh), in place into hbmt;
                # per m-subtile so DVE of one overlaps the out-DMA of the other
                for m in range(MSUB):
                    nc.vector.tensor_sub(
                        hbmt[:, m, :], hbmt[:, m, :], hh_sb[:, m, :]
                    )
                    nc.vector.tensor_mul(
                        hbmt[:, m, :], z_sb[:, m, :], hbmt[:, m, :]
                    )
                    nc.vector.tensor_add(
                        hbmt[:, m, :], hh_sb[:, m, :], hbmt[:, m, :]
                    )
                    nc.sync.dma_start(out_r[:, m, :], hbmt[:, m, :])

    nc.compile()
    return nc


def _pack_kmajor(a):
    # (BC, 2048) -> (128, 16, BC) with [p, o, b] = a[b, o*128+p]
    bc = a.shape[0]
    return np.ascontiguousarray(a.T.reshape(KCH, P, bc).transpose(1, 0, 2))


def make_in_maps(
    inputs, h_tm1, real_kernel, imaginary_kernel,
    real_recurrent_kernel, imaginary_recurrent_kernel, real_bias,
    imaginary_bias,
):
    inputs = np.ascontiguousarray(inputs, dtype=np.float32)
    h_tm1 = np.ascontiguousarray(h_tm1, dtype=np.float32)
    ws = {
        "RK": np.ascontiguousarray(real_kernel, dtype=np.float32),
        "IK": np.ascontiguousarray(imaginary_kernel, dtype=np.float32),
        "RR": np.ascontiguousarray(real_recurrent_kernel, dtype=np.float32),
        "IR": np.ascontiguousarray(imaginary_recurrent_kernel, dtype=np.float32),
    }
    rb = np.asarray(real_bias, dtype=np.float32)
    ib = np.asarray(imaginary_bias, dtype=np.float32)

    wsl = {}
    bsl = {}
    brc = {}
    for p in range(2):
        cols = [slice(g * U + p * UC, g * U + (p + 1) * UC) for g in range(3)]
        wsl[p] = {
            k: np.ascontiguousarray(np.concatenate([v[:, c] for c in cols], axis=1))
            for k, v in ws.items()
        }
        bsl[p] = np.concatenate([np.concatenate([rb[c], ib[c]]) for c in cols])
        # gate-r column-major bias, pre-scaled: [128, 16], [pp, ccg]
        br = np.concatenate([rb[cols[1]], ib[cols[1]]])  # (2048,) re|im own
        brc[p] = np.ascontiguousarray(
            (0.2 * br + 0.5).reshape(KCH, P).T
        )

    in_maps = []
    for c in range(N_CORES):
        g, p = c // 2, c % 2
        rs = slice(g * BC, (g + 1) * BC)
        ocr = slice(p * UC, (p + 1) * UC)
        oci = slice(U + p * UC, U + (p + 1) * UC)
        hbm = np.ascontiguousarray(
            np.concatenate([h_tm1[rs, ocr], h_tm1[rs, oci]], axis=1)
        )
        in_maps.append(
            {
                "x1": _pack_kmajor(inputs[rs, :U]),
                "x2": _pack_kmajor(inputs[rs, U:]),
                "h1": _pack_kmajor(h_tm1[rs, :U]),
                "h2": _pack_kmajor(h_tm1[rs, U:]),
                "hbm": hbm,
                "hTo": _pack_kmajor(hbm),
                "RK": wsl[p]["RK"],
                "IK": wsl[p]["IK"],
                "RR": wsl[p]["RR"],
                "IR": wsl[p]["IR"],
                "bias": bsl[p],
                "biasr": brc[p],
            }
        )
    return in_maps


def scatter_out(results):
    h_new = np.empty((B, 2 * U), dtype=np.float32)
    for c in range(N_CORES):
        g, p = c // 2, c % 2
        rs = slice(g * BC, (g + 1) * BC)
        o = results[c]["out"]
        h_new[rs, p * UC : (p + 1) * UC] = o[:, :UC]
        h_new[rs, U + p * UC : U + (p + 1) * UC] = o[:, UC:]
    return h_new


def _build_nc_retry(repeat=1, attempts=4):
    # Tile's scheduler very occasionally reports a spurious deadlock on a
    # valid graph (ordering is not fully deterministic); retry a few times.
    last = None
    for _ in range(attempts):
        try:
            return _build_nc(repeat=repeat)
        except Exception as e:  # noqa: BLE001
            if "Deadlock" not in type(e).__name__ + str(e):
                raise
            last = e
    raise last


def kernel(
    inputs,
    h_tm1,
    real_kernel,
    imaginary_kernel,
    real_recurrent_kernel,
    imaginary_recurrent_kernel,
    real_bias,
    imaginary_bias,
):
    if "nc" not in _CACHE:
        _CACHE["nc"] = _build_nc_retry()
    nc = _CACHE["nc"]
    in_maps = make_in_maps(
        inputs, h_tm1, real_kernel, imaginary_kernel,
        real_recurrent_kernel, imaginary_recurrent_kernel, real_bias,
        imaginary_bias,
    )
    res = run_bass_kernel_spmd(nc, in_maps, core_ids=list(range(N_CORES)))
    return scatter_out(res.results)



# revision 28
# speedup vs baseline: 1.7535x; 1.7535x over previous
"""CGRUCell Trainium2 kernel — 2 batch-groups x 4-way unit split on 8
NeuronCores, Gauss 3-multiplication complex matmuls, mixed precision.

Reference semantics (conjugate cat form), per gate with weights (W, V):
  preact_real = Xr@Wr + Xi@Wi + Hr@Vr + Hi@Vi
  preact_imag = Xi@Wr - Xr@Wi + Hi@Vr - Hr@Vi
Gauss: P1 = Xr@Wr + Hr@Vr; P2 = Xi@Wi + Hi@Vi;
       P3 = (Xi-Xr)@(Wr+Wi) + (Hi-Hr)@(Vr+Vi)
       real = P1+P2, imag = P3+P1-P2  (3 matmul streams instead of 4).

Core c: batch group g=c//4 (512 rows), unit quarter p=c%4 (unit cols
[p*512:(p+1)*512] real + matching imag). Per-core MACs 9.7e9 vs 1.29e10
non-Gauss.

Precision (validated vs reference in numpy): gate r fp8e4m3 DoubleRow
(error attenuated by hard_sigmoid's 0.2 slope then averaged by the
candidate matmul; rel err 1.6e-2 vs the 2e-2 gate); gates z, h fp16
(6e-4). R_FP8=False falls back to fp16 everywhere.

Gate r runs output-transposed (weights stationary, acts moving) so r*h
emerges K-major for the 4-way AllGather feeding gate h's recurrent side.
Gates z/h run batch-major (acts stationary, weights moving) in two
m-pair passes (Gauss needs 3 PSUM banks per output tile; 8-bank PSUM
fits 6 = 3 kinds x 2 m-subtiles, so weights stream twice). Gate order
r -> z -> h hides the collective under z. hard_sigmoid runs on the
scalar engine as w = Relu(1 - Relu(0.2y + b')) (= 1-z; blend uses
h - w*(h-hh)).
"""

import sys

for _p in ("/opt/trn_rl_repo", "/root/.axon_site/_ro/trn_rl_repo"):
    if _p not in sys.path:
        sys.path.append(_p)

import numpy as np
import ml_dtypes

import concourse.bass as bass
import concourse.mybir as mybir
import concourse.tile as tile
from concourse import bacc
from concourse.bass_utils import run_bass_kernel_spmd

P = 128
U = 2048            # UNITS
B = 1024
N_CORES = 8
GROUPS = 2          # batch groups
NPAR = 4            # unit-split ways
BC = B // GROUPS    # 512 batch rows per core
MSUB = BC // P      # 4 m-subtiles
UC = U // NPAR      # 512 own unit columns (per complex half)
CCH = UC // P       # 4 col chunks of 128
KCH = U // P        # 16 k-chunks per complex half
F32 = mybir.dt.float32
F16 = mybir.dt.float16
F8 = mybir.dt.float8e4
NF16 = np.float16
NF8 = ml_dtypes.float8_e4m3

R_FP8 = True        # gate r in fp8 DoubleRow (False -> fp16, safer/slower)

_CACHE = {}


def _build_nc(repeat=1, loop_iters=1):
    nc = bacc.Bacc(None, target_bir_lowering=False)
    AF = mybir.ActivationFunctionType
    DR = mybir.MatmulPerfMode.DoubleRow

    rdt = F8 if R_FP8 else F16
    # K-major activations [p, o, b]: value [o*128+p, b] of act.T
    xr8 = nc.dram_tensor("xr8", [P, KCH, BC], rdt, kind="ExternalInput")
    xi8 = nc.dram_tensor("xi8", [P, KCH, BC], rdt, kind="ExternalInput")
    xd8 = nc.dram_tensor("xd8", [P, KCH, BC], rdt, kind="ExternalInput")
    hr8 = nc.dram_tensor("hr8", [P, KCH, BC], rdt, kind="ExternalInput")
    hi8 = nc.dram_tensor("hi8", [P, KCH, BC], rdt, kind="ExternalInput")
    hd8 = nc.dram_tensor("hd8", [P, KCH, BC], rdt, kind="ExternalInput")
    xr16 = nc.dram_tensor("xr16", [P, KCH, BC], F16, kind="ExternalInput")
    xi16 = nc.dram_tensor("xi16", [P, KCH, BC], F16, kind="ExternalInput")
    hr16 = nc.dram_tensor("hr16", [P, KCH, BC], F16, kind="ExternalInput")
    hi16 = nc.dram_tensor("hi16", [P, KCH, BC], F16, kind="ExternalInput")
    # h own cols transposed K-major [p, cc(4 re + 4 im), b] fp16
    hT16 = nc.dram_tensor("hT16", [P, 2 * CCH, BC], F16, kind="ExternalInput")
    # h own cols batch-major [p, m, c(512 re | 512 im)] fp16
    hbm16 = nc.dram_tensor("hbm16", [P, MSUB, 2 * UC], F16,
                           kind="ExternalInput")
    wnames = ["kr", "ki", "ks", "rr", "ri", "rs"]
    wr = {n: nc.dram_tensor(f"wr_{n}", [P, KCH, UC], rdt,
                            kind="ExternalInput") for n in wnames}
    wz = {n: nc.dram_tensor(f"wz_{n}", [P, KCH, UC], F16,
                            kind="ExternalInput") for n in wnames}
    wh = {n: nc.dram_tensor(f"wh_{n}", [P, KCH, UC], F16,
                            kind="ExternalInput") for n in wnames}
    # biases [2, UC]: row0 = real bias (z ships b+2.5, hard-sigmoid
    # fold), row1 = b_im - b_re (for the saved imag diff); r pre-scaled
    # 0.2b+0.5 column-major [p, cc]
    bz = nc.dram_tensor("bz", [2, UC], F32, kind="ExternalInput")
    bh = nc.dram_tensor("bh", [2, UC], F32, kind="ExternalInput")
    brT = nc.dram_tensor("brT", [P, 2 * CCH], F32, kind="ExternalInput")
    out = nc.dram_tensor("out", [BC, 2 * UC], F32, kind="ExternalOutput")
    out_r = out.rearrange("(m p) c -> p m c", p=P)

    with tile.TileContext(nc) as tc:
        with (
            tc.tile_pool(name="acts", bufs=1) as acts,
            tc.tile_pool(name="psum", bufs=1, space="PSUM") as psum,
            tc.tile_pool(name="small", bufs=1) as small,
            tc.tile_pool(name="yp", bufs=2) as yp,
            tc.tile_pool(name="dram", bufs=1, space="DRAM") as dram,
        ):
            bt = {}
            for gn, src in (("z", bz), ("h", bh)):
                for half in range(2):
                    t = small.tile([P, UC], F32, tag=f"bt{gn}{half}",
                                   name=f"bt{gn}{half}")
                    nc.sync.dma_start(
                        t[:], src[None, half, :].to_broadcast((P, UC))
                    )
                    bt[(gn, half)] = t
            brTs = small.tile([P, 2 * CCH], F32, tag="brTs", name="brTs")
            nc.sync.dma_start(brTs[:], brT[:])
            a8 = {}
            a16 = {}
            rws = {}
            hTs = None

            def load_acts():
                # ---- gate r critical-path loads first (queue priority),
                # then z/h prefetches (behind r's loads in the queues) ----
                for nm in ("xr8", "xi8", "xd8", "hr8", "hi8", "hd8"):
                    a8[nm] = acts.tile([P, KCH, BC], rdt, tag=nm, name=nm)
                srcs = {"xr8": xr8, "xi8": xi8, "xd8": xd8, "hr8": hr8,
                        "hi8": hi8, "hd8": hd8}
                for o in range(0, KCH, 4):
                    sl = slice(o, o + 4)
                    for nm in srcs:
                        nc.sync.dma_start(a8[nm][:, sl, :],
                                          srcs[nm][:, sl, :])
                hTs_l = acts.tile([P, 2 * CCH, BC], F16, tag="hTs",
                                  name="hTs")
                nc.sync.dma_start(hTs_l[:], hT16[:])
                # r weights: full matrices in the shared weight tags w<n>
                # (8KB/partition slots, reused by z/h's fp16 stream chunks)
                for n in wnames:
                    rws[n] = acts.tile([P, KCH, UC], rdt, tag=f"w{n}",
                                       name=f"rw{n}")
                    nc.sync.dma_start(rws[n][:], wr[n][:])
                for nm, src in (("xr16", xr16), ("xi16", xi16),
                                ("hr16", hr16), ("hi16", hi16)):
                    a16[nm] = acts.tile([P, KCH, BC], F16, tag=nm, name=nm)
                    nc.sync.dma_start(a16[nm][:], src[:])
                return hTs_l

            def r_evict(ps_l, cc, rhTl, hTs):
                """rh_re/rh_im for col chunk cc from PSUM P1,P2,P3."""
                P1, P2, P3 = ps_l
                for half in range(2):
                    y = yp.tile([P, BC], F32, tag="y", name=f"y_r{cc}{half}")
                    if half == 0:
                        nc.vector.tensor_copy(y[:], P1[:])
                        nc.vector.tensor_add(y[:], y[:], P2[:])
                    else:
                        nc.vector.tensor_copy(y[:], P3[:])
                        nc.vector.tensor_add(y[:], y[:], P1[:])
                        nc.vector.tensor_sub(y[:], y[:], P2[:])
                    c = half * CCH + cc
                    a = yp.tile([P, BC], F32, tag="a", name=f"a_r{cc}{half}")
                    nc.scalar.activation(a[:], y[:], AF.Relu,
                                         bias=brTs[:, c : c + 1], scale=0.2)
                    u = yp.tile([P, BC], F16, tag="u", name=f"u_r{cc}{half}")
                    nc.scalar.activation(u[:], a[:], AF.Relu,
                                         bias=1.0, scale=-1.0)
                    # rh = hT - u*hT  (= r*h with r = 1-u)
                    nc.vector.tensor_mul(rhTl[:, c, :], u[:], hTs[:, c, :])
                    nc.vector.tensor_sub(rhTl[:, c, :], hTs[:, c, :],
                                         rhTl[:, c, :])

            def gate_r(rep, hTs):
                """fp8 DoubleRow (or fp16), output-transposed: PSUM
                [128 cols, BC] per col chunk; weights stationary."""
                rhTl = acts.tile([P, 2 * CCH, BC], F16, tag="rhTl",
                                 name=f"rhTl_{rep}")
                streams = [("kr", "xr8", 0), ("ki", "xi8", 1),
                           ("ks", "xd8", 2), ("rr", "hr8", 0),
                           ("ri", "hi8", 1), ("rs", "hd8", 2)]
                nk = KCH // 2 if R_FP8 else KCH
                for cc in range(CCH):
                    # alternate bank triples so cc+1's matmuls overlap
                    # cc's eviction
                    b0 = 3 * (cc % 2)
                    ps_l = [psum.tile([P, BC], F32, tag=f"ps{b0 + i}",
                                      name=f"psr_{rep}_{cc}_{i}")
                            for i in range(3)]
                    for k in range(nk):
                        for si, (wn, an, bank) in enumerate(streams):
                            if R_FP8:
                                lhsT = rws[wn][:, 2 * k : 2 * k + 2,
                                               cc * P : (cc + 1) * P]
                                rhs = a8[an][:, 2 * k : 2 * k + 2, :]
                            else:
                                lhsT = rws[wn][:, k, cc * P : (cc + 1) * P]
                                rhs = a8[an][:, k, :]
                            nc.tensor.matmul(
                                ps_l[bank], lhsT, rhs,
                                start=(k == 0 and si < 3),
                                stop=(k == nk - 1 and si >= 3),
                                perf_mode=DR if R_FP8 else None,
                            )
                    r_evict(ps_l, cc, rhTl, hTs)
                return rhTl

            def hs_or_tanh(dest_ap, y, gn, act_tanh):
                if act_tanh:
                    nc.scalar.activation(dest_ap, y[:], AF.Tanh)
                else:
                    a = yp.tile([P, UC], F32, tag="a", name=f"a{gn}")
                    nc.scalar.activation(a[:], y[:], AF.Relu, scale=0.2)
                    nc.scalar.activation(dest_ap, a[:], AF.Relu,
                                         bias=1.0, scale=-1.0)

            def gate_bm(rep, gn, wsrc, rstat, dest, act_tanh):
                """Batch-major gate: acts stationary fp16, weights
                moving, split by Gauss kind so every weight matrix
                streams exactly once. Pass A: P1 (kr+rr) and P2 (ki+ri)
                for all 4 m = 8 PSUM banks; evict real = P1+P2+b and
                save diff = b+P1-P2 to SBUF. Pass B: P3 (ks+rs), 4
                banks; imag = diff+P3."""
                r1, r2 = rstat
                QK = KCH // 4
                # one shared slot: z's dif is fully consumed by z's own
                # pass B before gate h allocates its dif
                dif = yp.tile([P, MSUB, UC], F16, tag="dif",
                              name=f"dif{gn}_{rep}", bufs=1)

                def wstream(names):
                    wt = {}
                    for n in names:
                        wt[n] = [
                            acts.tile([P, QK, UC], F16, tag=f"w{n}",
                                      name=f"w{gn}{n}_{rep}_{hk}")
                            for hk in range(4)
                        ]
                        for hk in range(4):
                            nc.sync.dma_start(
                                wt[n][hk][:],
                                wsrc[n][:, hk * QK : (hk + 1) * QK, :],
                            )
                    return wt

                # ---- pass A: kinds P1, P2 ----
                ps_l = [psum.tile([P, UC], F32, tag=f"ps{i}",
                                  name=f"psA{gn}_{rep}_{i}")
                        for i in range(8)]  # P1[m]=ps[m], P2[m]=ps[4+m]
                wt = wstream(["kr", "ki", "rr", "ri"])
                # x-side streams first: gate h's recurrent operands (the
                # gathered r*h) only gate the second half, so the
                # collective hides under the first
                for k in range(KCH):
                    hk, ko = divmod(k, QK)
                    for wn, stat, bank in (("kr", a16["xr16"], 0),
                                           ("ki", a16["xi16"], 4)):
                        for m in range(MSUB):
                            nc.tensor.matmul(
                                ps_l[bank + m],
                                stat[:, k, m * P : (m + 1) * P],
                                wt[wn][hk][:, ko, :],
                                start=(k == 0), stop=False,
                            )
                for k in range(KCH):
                    hk, ko = divmod(k, QK)
                    for wn, stat, bank in (("rr", r1, 0), ("ri", r2, 4)):
                        for m in range(MSUB):
                            nc.tensor.matmul(
                                ps_l[bank + m],
                                stat[:, k, m * P : (m + 1) * P],
                                wt[wn][hk][:, ko, :],
                                start=False, stop=(k == KCH - 1),
                            )
                for m in range(MSUB):
                    P1, P2 = ps_l[m], ps_l[4 + m]
                    t = yp.tile([P, UC], F32, tag="y", name=f"t{gn}_{m}")
                    nc.vector.tensor_add(t[:], bt[(gn, 0)][:], P1[:])
                    # diff = (b_im - b_re) + t - P2 = b_im + P1 - P2
                    d = yp.tile([P, UC], F32, tag="yd", name=f"d{gn}_{m}")
                    nc.vector.tensor_add(d[:], bt[(gn, 1)][:], t[:])
                    nc.vector.tensor_sub(dif[:, m, :], d[:], P2[:])
                    nc.vector.tensor_add(t[:], t[:], P2[:])
                    hs_or_tanh(dest[:, m, 0:UC], t, gn, act_tanh)

                # ---- pass B: kind P3 ----
                ps_b = [psum.tile([P, UC], F32, tag=f"ps{i}",
                                  name=f"psB{gn}_{rep}_{i}")
                        for i in range(MSUB)]
                wt = wstream(["ks", "rs"])
                for k in range(KCH):
                    hk, ko = divmod(k, QK)
                    xdk = yp.tile([P, BC], F16, tag="xdk",
                                  name=f"xd_{gn}{rep}{k}")
                    nc.vector.tensor_sub(xdk[:], a16["xi16"][:, k, :],
                                         a16["xr16"][:, k, :])
                    for m in range(MSUB):
                        nc.tensor.matmul(
                            ps_b[m], xdk[:, m * P : (m + 1) * P],
                            wt["ks"][hk][:, ko, :],
                            start=(k == 0), stop=False,
                        )
                for k in range(KCH):
                    hk, ko = divmod(k, QK)
                    rdk = yp.tile([P, BC], F16, tag="rdk",
                                  name=f"rd_{gn}{rep}{k}")
                    nc.vector.tensor_sub(rdk[:], r2[:, k, :],
                                         r1[:, k, :])
                    for m in range(MSUB):
                        nc.tensor.matmul(
                            ps_b[m], rdk[:, m * P : (m + 1) * P],
                            wt["rs"][hk][:, ko, :],
                            start=False, stop=(k == KCH - 1),
                        )
                for m in range(MSUB):
                    y = yp.tile([P, UC], F32, tag="y", name=f"yi{gn}_{m}")
                    nc.vector.tensor_add(y[:], dif[:, m, :], ps_b[m][:])
                    hs_or_tanh(dest[:, m, UC : 2 * UC], y, gn, act_tanh)

            from contextlib import nullcontext

            loop_cm = (tc.For_i(0, loop_iters) if loop_iters > 1
                       else nullcontext())
            with loop_cm:
              for rep in range(repeat):
                hTs = load_acts()
                # ---- gate r, then 4-way AllGather of rh (fp16) ----
                rhTl = gate_r(rep, hTs)
                inb = dram.tile([P, 2 * CCH, BC], F16, tag="inb",
                                name=f"inb_{rep}")
                outb = dram.tile([NPAR, P, 2 * CCH, BC], F16, tag="outb",
                                 name=f"outb_{rep}")
                nc.sync.dma_start(inb[:], rhTl[:])
                nc.gpsimd.collective_compute(
                    "AllGather",
                    mybir.AluOpType.bypass,
                    replica_groups=[[0, 1, 2, 3], [4, 5, 6, 7]],
                    ins=[inb[:].opt()],
                    outs=[outb[:].opt()],
                )
                # gathered rh K-major, aliased over hr16/hi16 (z is their
                # last reader, so the gather lands right after z)
                rhr = acts.tile([P, KCH, BC], F16, tag="hr16",
                                name=f"rhr_{rep}")
                rhi = acts.tile([P, KCH, BC], F16, tag="hi16",
                                name=f"rhi_{rep}")
                for q in range(NPAR):
                    qs = slice(q * CCH, (q + 1) * CCH)
                    nc.sync.dma_start(rhr[:, qs, :], outb[q, :, 0:CCH, :])
                    nc.sync.dma_start(rhi[:, qs, :],
                                      outb[q, :, CCH : 2 * CCH, :])

                # z/h output buffers alias fp8 act tiles (dead after r)
                w_sb = acts.tile([P, MSUB, 2 * UC], F16, tag="xi8",
                                 name=f"w_sb_{rep}")
                hh_sb = acts.tile([P, MSUB, 2 * UC], F16, tag="hr8",
                                  name=f"hh_sb_{rep}")
                hbms = acts.tile([P, MSUB, 2 * UC], F16, tag="xr8",
                                 name=f"hbms_{rep}")
                nc.sync.dma_start(hbms[:], hbm16[:])

                # ---- gate z (overlaps the collective) ----
                gate_bm(rep, "z", wz, (a16["hr16"], a16["hi16"]),
                        w_sb, act_tanh=False)
                # ---- gate h ----
                gate_bm(rep, "h", wh, (rhr, rhi), hh_sb,
                        act_tanh=True)

                # ---- blend h_new = h - w*(h - hh), per m ----
                for m in range(MSUB):
                    o1 = acts.tile([P, 2 * UC], F16, tag="hTs",
                                   name=f"o1_{rep}_{m}")
                    nc.vector.tensor_sub(o1[:], hbms[:, m, :],
                                         hh_sb[:, m, :])
                    nc.vector.tensor_mul(o1[:], w_sb[:, m, :], o1[:])
                    o2 = acts.tile([P, 2 * UC], F32, tag="rhTl",
                                   name=f"o2_{rep}_{m}")
                    nc.vector.tensor_sub(o2[:], hbms[:, m, :], o1[:])
                    nc.sync.dma_start(out_r[:, m, :], o2[:])

    nc.compile()
    return nc


def _pack_kmajor(a, dt):
    # (BC, K) -> (128, K//128, BC) with [p, o, b] = a[b, o*128+p]
    k = a.shape[1]
    return np.ascontiguousarray(
        a.T.reshape(k // P, P, a.shape[0]).transpose(1, 0, 2).astype(dt)
    )


def _pack_w(w, dt):
    # (2048, UC) -> (128, 16, UC) with [p, o, c] = w[o*128+p, c]
    return np.ascontiguousarray(
        w.reshape(KCH, P, UC).transpose(1, 0, 2).astype(dt)
    )


def make_in_maps(
    inputs, h_tm1, real_kernel, imaginary_kernel,
    real_recurrent_kernel, imaginary_recurrent_kernel, real_bias,
    imaginary_bias,
):
    x = np.ascontiguousarray(inputs, dtype=np.float32)
    h = np.ascontiguousarray(h_tm1, dtype=np.float32)
    rk = np.asarray(real_kernel, dtype=np.float32)
    ik = np.asarray(imaginary_kernel, dtype=np.float32)
    rr = np.asarray(real_recurrent_kernel, dtype=np.float32)
    ir = np.asarray(imaginary_recurrent_kernel, dtype=np.float32)
    rb = np.asarray(real_bias, dtype=np.float32)
    ib = np.asarray(imaginary_bias, dtype=np.float32)

    rdt = NF8 if R_FP8 else NF16
    Xr, Xi = x[:, :U], x[:, U:]
    Hr, Hi = h[:, :U], h[:, U:]
    Xd, Hd = Xi - Xr, Hi - Hr

    # per-parity weight/bias packs (shared by both batch groups)
    wpk = {}
    for p in range(NPAR):
        pk = {}
        for gn, dt in (("r", rdt), ("z", NF16), ("h", NF16)):
            g = {"z": 0, "r": 1, "h": 2}[gn]
            cs = slice(g * U + p * UC, g * U + (p + 1) * UC)
            pk[f"w{gn}_kr"] = _pack_w(rk[:, cs], dt)
            pk[f"w{gn}_ki"] = _pack_w(ik[:, cs], dt)
            pk[f"w{gn}_ks"] = _pack_w(rk[:, cs] + ik[:, cs], dt)
            pk[f"w{gn}_rr"] = _pack_w(rr[:, cs], dt)
            pk[f"w{gn}_ri"] = _pack_w(ir[:, cs], dt)
            pk[f"w{gn}_rs"] = _pack_w(rr[:, cs] + ir[:, cs], dt)
        zs = slice(p * UC, (p + 1) * UC)
        rs_ = slice(U + p * UC, U + (p + 1) * UC)
        hs_ = slice(2 * U + p * UC, 2 * U + (p + 1) * UC)
        pk["bz"] = np.stack([rb[zs] + 2.5, ib[zs] - rb[zs]]).astype(
            np.float32)
        pk["bh"] = np.stack([rb[hs_], ib[hs_] - rb[hs_]]).astype(
            np.float32)
        br = np.concatenate([rb[rs_], ib[rs_]])  # (1024,) re|im own
        pk["brT"] = np.ascontiguousarray(
            (0.2 * br + 0.5).reshape(2 * CCH, P).T.astype(np.float32)
        )
        wpk[p] = pk

    # per batch-group activation packs
    apk = {}
    for g in range(GROUPS):
        rows = slice(g * BC, (g + 1) * BC)
        apk[g] = {
            "xr8": _pack_kmajor(Xr[rows], rdt),
            "xi8": _pack_kmajor(Xi[rows], rdt),
            "xd8": _pack_kmajor(Xd[rows], rdt),
            "hr8": _pack_kmajor(Hr[rows], rdt),
            "hi8": _pack_kmajor(Hi[rows], rdt),
            "hd8": _pack_kmajor(Hd[rows], rdt),
            "xr16": _pack_kmajor(Xr[rows], NF16),
            "xi16": _pack_kmajor(Xi[rows], NF16),
            "hr16": _pack_kmajor(Hr[rows], NF16),
            "hi16": _pack_kmajor(Hi[rows], NF16),
        }

    in_maps = []
    for c in range(N_CORES):
        g, p = c // NPAR, c % NPAR
        rows = slice(g * BC, (g + 1) * BC)
        hcat = np.concatenate(
            [h[rows, p * UC : (p + 1) * UC],
             h[rows, U + p * UC : U + (p + 1) * UC]], axis=1
        )  # (BC, 1024) own re|im
        hT = np.ascontiguousarray(
            hcat.T.reshape(2 * CCH, P, BC).transpose(1, 0, 2).astype(NF16)
        )
        hbm = np.ascontiguousarray(
            hcat.reshape(MSUB, P, 2 * UC).transpose(1, 0, 2).astype(NF16)
        )
        m = {"hT16": hT, "hbm16": hbm}
        m.update(apk[g])
        m.update(wpk[p])
        in_maps.append(m)
    return in_maps


def scatter_out(results):
    h_new = np.empty((B, 2 * U), dtype=np.float32)
    for c in range(N_CORES):
        g, p = c // NPAR, c % NPAR
        rows = slice(g * BC, (g + 1) * BC)
        o = results[c]["out"]
        h_new[rows, p * UC : (p + 1) * UC] = o[:, :UC]
        h_new[rows, U + p * UC : U + (p + 1) * UC] = o[:, UC:]
    return h_new


def _build_nc_retry(repeat=1, loop_iters=1, attempts=4):
    # Tile's scheduler very occasionally reports a spurious deadlock on a
    # valid graph (ordering is not fully deterministic); retry a few times.
    last = None
    for _ in range(attempts):
        try:
            return _build_nc(repeat=repeat, loop_iters=loop_iters)
        except Exception as e:  # noqa: BLE001
            if "Deadlock" not in type(e).__name__ + str(e):
                raise
            last = e
    raise last


def kernel(
    inputs,
    h_tm1,
    real_kernel,
    imaginary_kernel,
    real_recurrent_kernel,
    imaginary_recurrent_kernel,
    real_bias,
    imaginary_bias,
):
    if "nc" not in _CACHE:
        _CACHE["nc"] = _build_nc_retry()
    nc = _CACHE["nc"]
    in_maps = make_in_maps(
        inputs, h_tm1, real_kernel, imaginary_kernel,
        real_recurrent_kernel, imaginary_recurrent_kernel, real_bias,
        imaginary_bias,
    )
    res = run_bass_kernel_spmd(nc, in_maps, core_ids=list(range(N_CORES)))
    return scatter_out(res.results)
